# revision 24
# baseline (speedup 1.0000x reference)
"""Causal self-attention (RoPE) Trainium2 Bass kernel for 8 NeuronCores.

Model: B=4, T=2048, C=1024, H=16 heads, head_dim=64.
Sharding: core c handles batch b=c//2 and head group hg=c%2 (8 heads each).
Each core computes a partial output y_c = attn_heads(b, hg) @ w_out[hg rows]
in f16; host sums the two partials per batch (the tensor-parallel
all-reduce).

Per-core pipeline (single NEFF, SPMD over 8 cores with different data):
  - v projection for all T tiles (f16 matmuls), v stored [T, hd] with a ones
    column appended per head (the softmax normalizer Z rides the AV matmul).
  - per head pair: q/k projection + RoPE (rotate-half via +-1 permutation
    matmul then cos/sin multiplies on DVE; q/k stored f16 transposed
    [hd, T]).  Projection of pair p+1 is emitted interleaved with the
    attention of pair p so PE and ACT overlap.
  - scores per head: scores^T = k^T q (K=64 matmuls into 2-bank PSUM
    tiles), exp on ACT (scale=1/8, f16, causally packed layout), diagonal
    causal mask multiply on DVE.
  - AV per head, v-stationary orientation: out^T[d, q-block] accumulates
    matmul(lhsT=v_aug[jk] (128 cols: 64 v cols | 64 ones cols, halves
    swapped for odd heads), rhs=exp[jk, q-block]) over key tiles jk.  The
    ones half makes 64 PSUM rows come out as the normalizer Z[q]
    replicated across 64 partitions for free, with the v rows landing on
    the same partition range outT[d] wants for that head's parity.
  - normalization (HW quirk: custom DVE ops only honor base partition 0,
    and DVE ops are only safe when all operands share a base): full-tile
    reciprocal_approx_fast PSUM->SBUF, one cross-base tensor_copy moves
    the 1/Z rows onto the head's partition range, one fused
    scalar_tensor_tensor multiply writes normalized outT[d, q] f16
    straight from PSUM.
  - y = outT^T @ w_out (f16 matmuls), DVE copy to f16 SBUF, DMA out.
"""

import math
from contextlib import ExitStack

import numpy as np

import concourse.bass as bass  # noqa: F401
import concourse.tile as tile
import concourse.mybir as mybir
from concourse import bacc

F32 = mybir.dt.float32
F16 = mybir.dt.float16
EXP = mybir.ActivationFunctionType.Exp

B, T, C = 4, 2048, 1024
H, HD = 16, 64
NCORES = 8
HPC = 8          # heads per core
NPAIR = 4        # head pairs per core
NKT = T // 128   # 16 k/q tiles of 128
KC = C // 128    # 8 contraction tiles for the qkv projection
NBLK = 4         # 512-wide query blocks
BW = T // NBLK   # 512

# exp_sb row offsets: row jk holds exp scores for keys [128jk, 128jk+128)
# x queries [128jk, T), packed contiguously.
EO = [0] * (NKT + 1)
for _jk in range(NKT):
    EO[_jk + 1] = EO[_jk] + (T - 128 * _jk)
EXP_COLS = EO[NKT]  # 17408


def _build_program():
    nc = bacc.Bacc("TRN2", target_bir_lowering=False, debug=False)

    # Single input blob (binding a DRAM tensor to the NEFF costs ~60us per
    # dispatch in this execution path, so ship ONE tensor, not five).
    # Layout (f16, 2048 cols):
    #   rows    0:1024  xT [C, T]
    #   rows 1024:1792  wqkv partition-major: row 1024+p*6+i, col j holds
    #                   wqkv_r[p, kc, n] at flat kc*1536+n = i*2048+j
    #   rows 1792:2048  wo partition-major, 2 rows of 2048 per partition
    #   rows 2048:2112  cs [64, T] (cos rows 0:32, sin rows 32:64)
    #   rows 2112:2120  prot [128,128] partition-major (16 partitions/row)
    #   rows 2120:2128  maskd [128,128] partition-major
    blob = nc.dram_tensor("blob", [2128, T], F16, kind="ExternalInput")
    y = nc.dram_tensor("y", [T, C], F16, kind="ExternalOutput")

    bap = blob.ap()
    xT_r = bap[0:1024, :].rearrange("(kc p) t -> p kc t", p=128)
    w_r = bap[1024:1792, :].rearrange(
        "(p s) w -> p (s w)", s=6).rearrange(
        "p (kc n) -> p kc n", n=1536)
    wo_r = bap[1792:2048, :].rearrange(
        "(p s) w -> p (s w)", s=2).rearrange(
        "p (kc n) -> p kc n", n=1024)
    cs_ap = bap[2048:2112, :]
    prot_ap = bap[2112:2120, :].rearrange("r (pb c) -> (r pb) c", pb=16)
    maskd_ap = bap[2120:2128, :].rearrange("r (pb c) -> (r pb) c", pb=16)
    y_ap = y.ap()

    with tile.TileContext(nc) as tc, ExitStack() as ctx:
        persist = ctx.enter_context(tc.tile_pool(name="persist", bufs=1))
        v_aug = persist.tile([128, NKT, NPAIR, 2, 128], F16)
        qrot = persist.tile([128, NPAIR, T], F16)
        krot = persist.tile([128, NPAIR, T], F16)
        outT = persist.tile([128, NPAIR, T], F16)
        wo_sb = persist.tile([128, 4, C], F16)
        maskd_sb = persist.tile([128, 128], F16)
        xT_sb = persist.tile([128, KC, T], F16)
        wq_sb = persist.tile([128, KC, 512], F16)
        wk_sb = persist.tile([128, KC, 512], F16)
        cos_sb = persist.tile([128, T], F16)
        sin_sb = persist.tile([128, T], F16)
        prot_sb = persist.tile([128, 128], F16)

        nc.vector.memset(v_aug[:], 1.0)
        for c in range(4):
            sl = slice(c * 512, (c + 1) * 512)
            nc.sync.dma_start(xT_sb[:, :, sl], xT_r[:, :, sl])
        nc.sync.dma_start(wq_sb[:], w_r[:, :, 0:512])
        nc.sync.dma_start(wk_sb[:], w_r[:, :, 512:1024])
        for g in range(4):
            nc.sync.dma_start(cos_sb[32 * g:32 * g + 32, :], cs_ap[0:32, :])
            nc.sync.dma_start(sin_sb[32 * g:32 * g + 32, :], cs_ap[32:64, :])
        nc.sync.dma_start(prot_sb[:], prot_ap)
        nc.sync.dma_start(wo_sb[:], wo_r)
        nc.sync.dma_start(maskd_sb[:], maskd_ap)

        with (
            tc.tile_pool(name="rawp", bufs=3) as rawp,
            tc.tile_pool(name="t1p", bufs=3) as t1p,
            tc.tile_pool(name="pps", bufs=1, space="PSUM") as pps,
            tc.tile_pool(name="ppse", bufs=1, space="PSUM") as ppse,
            tc.tile_pool(name="wvp", bufs=1) as wvp,
            tc.tile_pool(name="expp", bufs=1) as expp,
            tc.tile_pool(name="qkps", bufs=2, space="PSUM") as qkps,
            tc.tile_pool(name="avps", bufs=2, space="PSUM") as avps,
            tc.tile_pool(name="rzp", bufs=2) as rzp,
            tc.tile_pool(name="rzsp", bufs=2) as rzsp,
            tc.tile_pool(name="ysb", bufs=4) as ysbp,
        ):
            # ---- v projection for all T tiles ----
            wv_sb = wvp.tile([128, KC, 512], F16)
            nc.sync.dma_start(wv_sb[:], w_r[:, :, 1024:1536])
            for tt in range(NKT):
                ps = pps.tile([128, 512], F32, tag="ps_proj")
                for kc in range(KC):
                    nc.tensor.matmul(
                        ps[:],
                        xT_sb[:, kc, tt * 128:(tt + 1) * 128],
                        wv_sb[:, kc, :],
                        start=(kc == 0), stop=(kc == KC - 1),
                    )
                # two strided cast-copies; odd heads go to cols 64:128 so
                # their out^T rows land on partitions 64:128 in the AV.
                psr = ps[:].rearrange("p (h2 two d) -> p h2 two d", two=2, d=64)
                nc.vector.tensor_copy(v_aug[:, tt, :, 0, 0:64], psr[:, :, 0, :])
                nc.vector.tensor_copy(v_aug[:, tt, :, 1, 64:128], psr[:, :, 1, :])

            def emit_qk_proj(pr, w_sb, dst):
                """Projection + RoPE for one of q/k of pair pr (all T)."""
                for c in range(4):
                    sl = slice(c * 512, (c + 1) * 512)
                    ps = pps.tile([128, 512], F32, tag="ps_proj")
                    for kc in range(KC):
                        nc.tensor.matmul(
                            ps[:],
                            w_sb[:, kc, pr * 128:(pr + 1) * 128],
                            xT_sb[:, kc, sl],
                            start=(kc == 0), stop=(kc == KC - 1),
                        )
                    raw = rawp.tile([128, 512], F16)
                    nc.vector.tensor_copy(raw[:], ps[:])
                    ps2 = ppse.tile([128, 512], F32, tag="ps_rope")
                    nc.tensor.matmul(ps2[:], prot_sb[:], raw[:])
                    t1 = t1p.tile([128, 512], F16)
                    nc.vector.tensor_mul(t1[:], ps2[:], sin_sb[:, sl])
                    dsl = dst[:, pr, sl]
                    nc.vector.tensor_mul(dsl, raw[:], cos_sb[:, sl])
                    nc.vector.tensor_add(dsl, dsl, t1[:])

            def emit_qk_scores(h):
                pr, off = h // 2, (h % 2) * 64
                exp_sb = expp.tile([128, EXP_COLS], F16)
                for jk in range(NKT):
                    klhs = krot[off:off + 64, pr, jk * 128:(jk + 1) * 128]
                    wtot = T - jk * 128
                    for segoff in range(0, wtot, 1024):
                        segw = min(1024, wtot - segoff)
                        pss = qkps.tile([128, 1024], F32)
                        for si in range(math.ceil(segw / 512)):
                            sw = min(512, segw - si * 512)
                            q0 = jk * 128 + segoff + si * 512
                            nc.tensor.matmul(
                                pss[:, si * 512: si * 512 + sw],
                                klhs,
                                qrot[off:off + 64, pr, q0:q0 + sw],
                            )
                        e0 = EO[jk] + segoff
                        nc.scalar.activation(
                            exp_sb[:, e0:e0 + segw], pss[:, 0:segw],
                            EXP, scale=0.125,
                        )
                    nc.vector.tensor_mul(
                        exp_sb[:, EO[jk]:EO[jk] + 128],
                        exp_sb[:, EO[jk]:EO[jk] + 128],
                        maskd_sb[:],
                    )
                return exp_sb

            def emit_av(h, exp_sb):
                """V-stationary AV: outT[d, q] per 512-wide q block.

                psav rows [off, off+64) = out^T, rows [64-off, 128-off) =
                softmax normalizer Z[q] replicated (ones half of v_aug)."""
                off = (h % 2) * 64
                zoff = 64 - off
                for b in range(NBLK):
                    psav = avps.tile([128, BW], F32, tag="avt")
                    njk = 4 * b + 4   # jk = 0..4b+3 touch this block
                    for jk in range(njk):
                        if 128 * jk <= BW * b:
                            # full-width row
                            e0 = EO[jk] + (BW * b - 128 * jk)
                            nc.tensor.matmul(
                                psav[:, :],
                                v_aug[:, jk, h // 2, h % 2, :],
                                exp_sb[:, e0:e0 + BW],
                                start=(jk == 0), stop=(jk == njk - 1),
                            )
                        else:
                            # diagonal row: covers q in [128jk, 512(b+1))
                            w = BW * (b + 1) - 128 * jk
                            o = 128 * jk - BW * b
                            nc.tensor.matmul(
                                psav[:, o:o + w],
                                v_aug[:, jk, h // 2, h % 2, :],
                                exp_sb[:, EO[jk]:EO[jk] + w],
                                start=False, stop=(jk == njk - 1),
                            )
                    rzf = rzp.tile([128, BW], F32)
                    with nc.allow_low_precision(reason="softmax reciprocal"):
                        nc.vector.reciprocal_approx_fast(rzf[:], psav[:])
                    rzs = rzsp.tile([128, BW], F32)
                    nc.vector.tensor_copy(
                        rzs[off:off + 64, :], rzf[zoff:zoff + 64, :]
                    )
                    nc.vector.scalar_tensor_tensor(
                        outT[off:off + 64, h // 2, b * BW:(b + 1) * BW],
                        psav[off:off + 64, :], 1.0, rzs[off:off + 64, :],
                        mybir.AluOpType.mult, mybir.AluOpType.mult,
                    )

            def emit_out_proj(b):
                """y tiles of 512-wide query block b (all pairs' outT ready)."""
                for tt in range(4 * b, 4 * b + 4):
                    for nn in range(2):
                        psy = avps.tile([128, BW], F32, tag="avt")
                        for k4 in range(4):
                            nc.tensor.matmul(
                                psy[:],
                                outT[:, k4, tt * 128:(tt + 1) * 128],
                                wo_sb[:, k4, nn * 512:(nn + 1) * 512],
                                start=(k4 == 0), stop=(k4 == 3),
                            )
                        y_sb = ysbp.tile([128, 512], F16)
                        nc.scalar.copy(y_sb[:], psy[:])
                        nc.sync.dma_start(
                            y_ap[tt * 128:(tt + 1) * 128,
                                 nn * 512:(nn + 1) * 512],
                            y_sb[:],
                        )

            emit_qk_proj(0, wq_sb, qrot)
            emit_qk_proj(0, wk_sb, krot)

            for pr in range(NPAIR):
                exp0 = emit_qk_scores(2 * pr)
                if pr < NPAIR - 1:
                    emit_qk_proj(pr + 1, wq_sb, qrot)
                emit_av(2 * pr, exp0)

                exp1 = emit_qk_scores(2 * pr + 1)
                if pr < NPAIR - 1:
                    emit_qk_proj(pr + 1, wk_sb, krot)
                emit_av(2 * pr + 1, exp1)

                if pr == NPAIR - 1:
                    # interleave the output projection with the last head's
                    # AV blocks: block b's y only needs outT up to block b.
                    for b in range(NBLK):
                        emit_out_proj(b)

    nc.compile()
    return nc


def _host_tables():
    half = HD // 2
    theta = 1.0 / (10000.0 ** (np.arange(half, dtype=np.float64) / half))
    pos = np.arange(T, dtype=np.float64)
    freqs = np.outer(pos, theta)                      # [T, 32]
    cs = np.concatenate([np.cos(freqs).T, np.sin(freqs).T], 0)  # [64, T]

    prot = np.zeros((128, 128), np.float64)
    for b in (0, 64):
        for i in range(32):
            prot[b + 32 + i, b + i] = -1.0   # out[i] = -q[i+32]
            prot[b + i, b + 32 + i] = 1.0    # out[32+i] = +q[i]

    maskd = (np.arange(128)[None, :] >= np.arange(128)[:, None])  # f >= p
    return (cs.astype(np.float16), prot.astype(np.float16),
            maskd.astype(np.float16))


_NC_CACHE = []


def _get_program():
    if not _NC_CACHE:
        _NC_CACHE.append(_build_program())
    return _NC_CACHE[0]


def make_in_map(x, w_qkv, w_out, core, tables=None, xT_cache=None):
    if tables is None:
        tables = _host_tables()
    cs, prot, maskd = tables
    b, hg = core // 2, core % 2
    colsel = slice(hg * 512, (hg + 1) * 512)
    if xT_cache is not None and b in xT_cache:
        xTb = xT_cache[b]
    else:
        xTb = np.ascontiguousarray(x[b].T).astype(np.float16)
        if xT_cache is not None:
            xT_cache[b] = xTb
    wq = w_qkv[:, 0 * C:1 * C][:, colsel]
    wk = w_qkv[:, 1 * C:2 * C][:, colsel]
    wv = w_qkv[:, 2 * C:3 * C][:, colsel]
    wqkv = np.concatenate([wq, wk, wv], axis=1).astype(np.float16)
    wo_hg = w_out[colsel, :].astype(np.float16)

    blob = np.empty((2128, T), np.float16)
    blob[0:1024] = xTb
    blob[1024:1792] = (
        wqkv.reshape(8, 128, 1536).transpose(1, 0, 2).reshape(768, 2048))
    blob[1792:2048] = (
        wo_hg.reshape(4, 128, 1024).transpose(1, 0, 2).reshape(256, 2048))
    blob[2048:2112] = cs
    blob[2112:2120] = prot.reshape(8, 2048)
    blob[2120:2128] = maskd.reshape(8, 2048)
    return {"blob": blob}


_EXEC_CACHE = {}

# unique-data pack layout (rows of a [6224, 2048] f16 host array):
#   0:4096     xT of the 4 batches (1024 rows each)
#   4096:5632  wqkv partition-major, 2 head-group variants (768 rows each)
#   5632:6144  wo partition-major, 2 head-group variants (256 rows each)
#   6144:6224  tables: cs 64 rows, prot 8, maskd 8
U_ROWS = 6224


def _get_sharded_exec():
    """Build (once) the jitted assemble/exec/reduce pipeline + dev zeros.

    Host->device upload through the axon tunnel is slow (~30 MB/s), so
    kernel() uploads only the 24.3 MB of UNIQUE data, sharded 1/8th per
    core; an on-device all_gather + slice (jit1) materializes each core's
    8.7 MB input blob over the fast on-chip links, the bass NEFF runs
    (jit2), and an on-device pair-wise psum + halving (jit3) implements
    the tensor-parallel all-reduce so only 16 MB of final output is
    downloaded."""
    if _EXEC_CACHE:
        return _EXEC_CACHE["v"]
    import jax
    from jax.sharding import Mesh, PartitionSpec, NamedSharding
    from jax.experimental.shard_map import shard_map
    from concourse import bass2jax

    bass2jax.install_neuronx_cc_hook()
    nc = _get_program()

    in_names, out_names, out_avals = [], [], []
    for alloc in nc.m.functions[0].allocations:
        if not isinstance(alloc, mybir.MemoryLocationSet):
            continue
        name = alloc.memorylocations[0].name
        if alloc.kind == "ExternalInput":
            if name == "partition_id":
                continue
            in_names.append(name)
        elif alloc.kind == "ExternalOutput":
            out_names.append(name)
            shape = tuple(alloc.tensor_shape)
            dtype = mybir.dt.np(alloc.dtype)
            out_avals.append(jax.core.ShapedArray(shape, dtype))
    n_params = len(in_names)
    part_name = nc.partition_id_tensor.name if nc.partition_id_tensor else None
    all_in = in_names + out_names + ([part_name] if part_name else [])

    def _body(*args):
        operands = list(args)
        if part_name:
            operands = operands + [bass2jax.partition_id_tensor()]
        outs = bass2jax._bass_exec_p.bind(
            *operands,
            out_avals=tuple(out_avals),
            in_names=tuple(all_in),
            out_names=tuple(out_names),
            lowering_input_output_aliases=(),
            sim_require_finite=True,
            sim_require_nnan=True,
            nc=nc,
        )
        return tuple(outs)

    devices = jax.devices()[:NCORES]
    mesh = Mesh(np.asarray(devices), ("core",))
    spec = PartitionSpec("core")
    n_outs = len(out_names)
    sharded = jax.jit(
        shard_map(_body, mesh=mesh, in_specs=(spec,) * (n_params + n_outs),
                  out_specs=(spec,) * n_outs, check_rep=False),
        keep_unused=True,
    )
    sharding = NamedSharding(mesh, spec)
    # output scratch buffers created on-device (no host->device upload)
    dev_zero = [
        jax.jit(lambda av=av: jax.numpy.zeros(
            (NCORES * av.shape[0], *av.shape[1:]), av.dtype),
            out_shardings=sharding)()
        for av in out_avals
    ]
    jax.block_until_ready(dev_zero)

    def _assemble(u):
        full = jax.lax.all_gather(u, "core", tiled=True)   # [6224, 2048]
        cid = jax.lax.axis_index("core")
        xpart = jax.lax.dynamic_slice_in_dim(full, (cid // 2) * 1024, 1024, 0)
        wpart = jax.lax.dynamic_slice_in_dim(
            full, 4096 + (cid % 2) * 768, 768, 0)
        wopart = jax.lax.dynamic_slice_in_dim(
            full, 5632 + (cid % 2) * 256, 256, 0)
        tabs = jax.lax.slice_in_dim(full, 6144, 6224, axis=0)
        return jax.numpy.concatenate([xpart, wpart, wopart, tabs], 0)

    assemble = jax.jit(
        shard_map(_assemble, mesh=mesh, in_specs=(spec,),
                  out_specs=spec, check_rep=False))

    def _pairsum(y):
        s = jax.lax.psum(
            y, "core", axis_index_groups=[[0, 1], [2, 3], [4, 5], [6, 7]])
        cid = jax.lax.axis_index("core")
        return jax.lax.dynamic_slice_in_dim(s, (cid % 2) * 1024, 1024, 0)

    pairsum = jax.jit(
        shard_map(_pairsum, mesh=mesh, in_specs=(spec,),
                  out_specs=spec, check_rep=False))

    _EXEC_CACHE["v"] = (sharded, sharding, in_names, out_names, out_avals,
                        dev_zero, assemble, pairsum)
    return _EXEC_CACHE["v"]


def _pack_unique(x, w_qkv, w_out, tables):
    """Pack the unique per-call data into one [6224, 2048] f16 array."""
    cs, prot, maskd = tables
    U = np.empty((U_ROWS, T), np.float16)
    for b in range(B):
        U[b * 1024:(b + 1) * 1024] = x[b].T
    for hg in range(2):
        colsel = slice(hg * 512, (hg + 1) * 512)
        wq = w_qkv[:, 0 * C:1 * C][:, colsel]
        wk = w_qkv[:, 1 * C:2 * C][:, colsel]
        wv = w_qkv[:, 2 * C:3 * C][:, colsel]
        wqkv = np.concatenate([wq, wk, wv], axis=1).astype(np.float16)
        U[4096 + hg * 768:4096 + (hg + 1) * 768] = (
            wqkv.reshape(8, 128, 1536).transpose(1, 0, 2).reshape(768, 2048))
        wo_hg = w_out[colsel, :].astype(np.float16)
        U[5632 + hg * 256:5632 + (hg + 1) * 256] = (
            wo_hg.reshape(4, 128, 1024).transpose(1, 0, 2).reshape(256, 2048))
    U[6144:6208] = cs
    U[6208:6216] = prot.reshape(8, 2048)
    U[6216:6224] = maskd.reshape(8, 2048)
    return U


def kernel(x, w_qkv, w_out):
    x = np.asarray(x, dtype=np.float32)
    w_qkv = np.asarray(w_qkv, dtype=np.float32)
    w_out = np.asarray(w_out, dtype=np.float32)
    tables = _host_tables()

    from concourse.bass_utils import axon_active
    if not axon_active():
        from concourse.bass_utils import run_bass_kernel_spmd
        in_maps = [make_in_map(x, w_qkv, w_out, core, tables, {})
                   for core in range(NCORES)]
        res = run_bass_kernel_spmd(_get_program(), in_maps,
                                   core_ids=list(range(NCORES)))
        out = np.zeros((B, T, C), np.float32)
        for core in range(NCORES):
            out[core // 2] += res.results[core]["y"].astype(np.float32)
        return out

    (sharded, sharding, in_names, out_names, out_avals, dev_zero,
     assemble, pairsum) = _get_sharded_exec()
    U = _pack_unique(x, w_qkv, w_out, tables)
    blob = assemble(U)
    outs = sharded(blob, *dev_zero)
    yhalf = pairsum(outs[out_names.index("y")])
    res = np.asarray(yhalf).reshape(NCORES, 1024, C).astype(np.float32)
    out = np.empty((B, T, C), np.float32)
    for b in range(B):
        out[b, 0:1024] = res[2 * b]
        out[b, 1024:2048] = res[2 * b + 1]
    return out
